# revision 28
# baseline (speedup 1.0000x reference)
"""Trainium2 Bass kernel for nn_DepthAwareProjector.

Two SPMD launches on 8 NeuronCores:
  Phase 1 (per-camera nets): 24 half-images (12 cams x 2 halves) -> 3 per core.
    depth branch: conv3x3(256->256)+BN+ReLU -> conv1x1(256->41) = depth_logits
    ctx branch:   conv3x3(256->256)+BN+ReLU -> conv1x1(256->32) = context
    (softmax-sum over depth bins == 1, so pooled == context)
  Phase 2 (bev): bilinear 32x88 -> 400x400 fused with conv3x3(192->64),
    gelu, conv1x1(64->64). Sharded (batch 2 x 4 row-blocks of 100) -- blocks
    align with the period-25 interp phase pattern so all cores run one program.

All matmuls in float32r (full PE rate at N>=256, ~1.5e-4 rel err).
"""
import sys
if '/opt/trn_rl_repo' not in sys.path:
    sys.path.insert(0, '/opt/trn_rl_repo')

import numpy as np
import concourse.bass as bass
import concourse.tile as tile
from concourse import bacc, mybir
from concourse import bass_utils

F32 = mybir.dt.float32
F32R = mybir.dt.float32r
AF = mybir.ActivationFunctionType
ALU = mybir.AluOpType

N_CORES = 8
B6, CIN, H, W = 12, 256, 32, 88
DB, CTX = 41, 32
BEV, O1, CB = 400, 64, 192   # bev dim, compressor out ch, compressor in ch
NJ = 12                      # pooled-window rows per core (jrel 0..11)

# ---------------------------------------------------------------- schedule --
def _s_of(rr):
    return (rr + 0.5) * 0.08 - 0.5

def _cell_of(rr):
    return int(np.floor(_s_of(rr)))

def _schedule():
    """Row-pair classification for a 100-row block (block-independent)."""
    interior, boundary = [], []
    for i in range(50):
        cells = {_cell_of(2 * i + q + d - 1) for q in (0, 1) for d in range(3)}
        if len(cells) == 1 and i not in (0, 49):
            interior.append((i, min(cells)))
        else:
            js = sorted({c for c in cells} | {c + 1 for c in cells})
            boundary.append((i, js))
    return interior, boundary

INTERIOR, BOUNDARY = _schedule()
CELLS = sorted({a for _, a in INTERIOR})
PAIRS_OF_CELL = {a: [i for i, aa in INTERIOR if aa == a] for a in CELLS}
# boundary slot layout: per pair, slots = [(kt, t, j) for j in js for t in 3 for kt in 2]
B_SLOTS = [[(kt, t, j) for j in js for t in range(3) for kt in range(2)]
           for _, js in BOUNDARY]
B_OFF = np.cumsum([0] + [len(s) for s in B_SLOTS]).tolist()
TOT_BSLOTS = B_OFF[-1]

def _interp_matrix(n_src, n_dst):
    M = np.zeros((n_dst, n_src), np.float32)
    for d in range(n_dst):
        s = (d + 0.5) * n_src / n_dst - 0.5
        s0 = int(np.floor(s)); t = s - s0
        M[d, np.clip(s0, 0, n_src - 1)] += 1 - t
        M[d, np.clip(s0 + 1, 0, n_src - 1)] += t
    return M

def _round_f32r(a):
    """Round to fp32r (11-bit mantissa, RNE) so HWDGE DMAs need no cast."""
    b = np.ascontiguousarray(a, np.float32).view(np.uint32)
    drop = 12
    half = np.uint32(1 << (drop - 1))
    mask = np.uint32((1 << drop) - 1)
    low = b & mask
    r = b & ~mask
    add = (low > half) | ((low == half) & (((b >> drop) & 1) == 1))
    return (r + (add.astype(np.uint32) << drop)).view(np.float32)

# ------------------------------------------------------------------ phase 1 --
def build_phase1():
    nc = bacc.Bacc("TRN2", target_bir_lowering=False, debug=False,
                   enable_asserts=True, num_devices=N_CORES)
    feat = nc.dram_tensor("feat", [3, 2, 128, 18, 90], F32R, kind="ExternalInput").ap()
    w1 = nc.dram_tensor("w1", [2, 2, 128, 9 * 256], F32R, kind="ExternalInput").ap()
    b1 = nc.dram_tensor("b1", [128, 4], F32, kind="ExternalInput").ap()
    w2d = nc.dram_tensor("w2d", [2, 128, DB], F32R, kind="ExternalInput").ap()
    w2c = nc.dram_tensor("w2c", [2, 128, CTX], F32R, kind="ExternalInput").ap()
    b2d = nc.dram_tensor("b2d", [DB, 1], F32, kind="ExternalInput").ap()
    b2c = nc.dram_tensor("b2c", [CTX, 1], F32, kind="ExternalInput").ap()
    dl_out = nc.dram_tensor("dl_out", [3, DB, 16, 88], F32, kind="ExternalOutput").ap()
    ctx_out = nc.dram_tensor("ctx_out", [3, CTX, 16, 88], F32, kind="ExternalOutput").ap()

    with tile.TileContext(nc) as tc:
        with tc.tile_pool(name="wp", bufs=1) as wp, \
             tc.tile_pool(name="fp", bufs=2) as fp, \
             tc.tile_pool(name="hp", bufs=2) as hp, \
             tc.tile_pool(name="op", bufs=2) as op, \
             tc.tile_pool(name="ps", bufs=6, space="PSUM") as ps, \
             tc.tile_pool(name="ps2", bufs=2, space="PSUM") as ps2:

            # feat for half 0 first so the first matmul group starts early
            ft0 = []
            for kt in range(2):
                t_ = fp.tile([128, 18, 90], F32R, tag=f"f{kt}", name=f"f{kt}")
                nc.sync.dma_start(t_[:], feat[0, kt, :, :, :])
                ft0.append(t_)
            w1ts = {}
            for br in range(2):
                for kt in range(2):
                    t_ = wp.tile([128, 9 * 256], F32R, tag=f"w1{br}{kt}",
                                 name=f"w1{br}{kt}")
                    for c3 in range(3):
                        nc.scalar.dma_start(t_[:, c3 * 768:(c3 + 1) * 768],
                                            w1[br, kt, :, c3 * 768:(c3 + 1) * 768])
                    w1ts[br, kt] = t_
            w2dt = wp.tile([128, 2, DB], F32R, tag="w2d")
            w2ct = wp.tile([128, 2, CTX], F32R, tag="w2c")
            for kt in range(2):
                nc.sync.dma_start(w2dt[:, kt, :], w2d[kt, :, :])
                nc.sync.dma_start(w2ct[:, kt, :], w2c[kt, :, :])
            b1t = wp.tile([128, 4], F32, tag="b1")
            nc.sync.dma_start(b1t[:], b1[:])
            b2dt = wp.tile([DB, 1], F32, tag="b2d")
            nc.sync.dma_start(b2dt[:], b2d[:])
            b2ct = wp.tile([CTX, 1], F32, tag="b2c")
            nc.sync.dma_start(b2ct[:], b2c[:])

            for half in range(3):
                if half == 0:
                    ft = ft0
                else:
                    ft = []
                    for kt in range(2):
                        t_ = fp.tile([128, 18, 90], F32R, tag=f"f{kt}",
                                     name=f"f{kt}")
                        nc.sync.dma_start(t_[:], feat[half, kt, :, :, :])
                        ft.append(t_)
                for br in range(2):
                    ht = [hp.tile([128, 16, 88], F32R, tag=f"h{br}{m}", name=f"h{br}{m}")
                          for m in range(2)]
                    if br == 0:
                        w2t, b2t, no, dst = w2dt, b2dt, DB, dl_out
                    else:
                        w2t, b2t, no, dst = w2ct, b2ct, CTX, ctx_out
                    st = op.tile([no, 16, 88], F32, tag=f"o{br}", name=f"o{br}")
                    for chunk in range(4):
                        for m in range(2):
                            pst = ps.tile([128, 4, 88], F32, tag="acc")
                            n = 0
                            for kt in range(2):
                                for dy in range(3):
                                    for dx in range(3):
                                        nc.tensor.matmul(
                                            pst[:],
                                            w1ts[br, kt][:,
                                                (dy * 3 + dx) * 256 + m * 128:
                                                (dy * 3 + dx) * 256 + m * 128 + 128],
                                            ft[kt][:, chunk * 4 + dy:chunk * 4 + dy + 4,
                                                   dx:dx + 88],
                                            start=(n == 0), stop=(n == 17))
                                        n += 1
                            nc.scalar.activation(
                                ht[m][:, chunk * 4:chunk * 4 + 4, :], pst[:],
                                AF.Relu, bias=b1t[:, br * 2 + m:br * 2 + m + 1])
                        p2 = ps2.tile([128, 4, 88], F32, tag="acc2")
                        for kt in range(2):
                            nc.tensor.matmul(
                                p2[0:no, :, :], w2t[:, kt, :],
                                ht[kt][:, chunk * 4:chunk * 4 + 4, :],
                                start=(kt == 0), stop=(kt == 1))
                        nc.scalar.activation(
                            st[:, chunk * 4:chunk * 4 + 4, :], p2[0:no, :, :],
                            AF.Identity, bias=b2t[:])
                    nc.sync.dma_start(dst[half], st[:])
    nc.compile()
    return nc

# ---------------------------------------------------------------- host prep --
def _fold_bn(w, b, gamma, beta, mean, var, eps=1e-5):
    sc = gamma / np.sqrt(var + eps)
    return (w * sc[:, None, None, None]).astype(np.float32), \
           (b * sc + (beta - mean * sc)).astype(np.float32)

def _phase1_inputs(features, dw1, db1, dgamma, dbeta, dmean, dvar, dw2, db2,
                   cw1, cb1, cgamma, cbeta, cmean, cvar, cw2, cb2):
    dw1f, db1f = _fold_bn(dw1, db1, dgamma, dbeta, dmean, dvar)
    cw1f, cb1f = _fold_bn(cw1, cb1, cgamma, cbeta, cmean, cvar)

    w1 = np.zeros((2, 2, 128, 9 * 256), np.float32)
    for br, wf in ((0, dw1f), (1, cw1f)):
        # lhsT[c_in, o] per tap; layout [kt, c, tap*256 + o]
        t_ = wf.transpose(1, 2, 3, 0).reshape(256, 9, 256)  # [c_in, tap, o]
        for kt in range(2):
            w1[br, kt] = t_[kt * 128:(kt + 1) * 128].reshape(128, 9 * 256)
    b1 = np.zeros((128, 4), np.float32)
    for br, bf in ((0, db1f), (1, cb1f)):
        for m in range(2):
            b1[:, br * 2 + m] = bf[m * 128:(m + 1) * 128]
    w2dh = dw2[:, :, 0, 0].T.reshape(2, 128, DB).astype(np.float32)
    w2ch = cw2[:, :, 0, 0].T.reshape(2, 128, CTX).astype(np.float32)

    feat_cores = []
    for c in range(N_CORES):
        fw = np.zeros((3, 256, 18, 90), np.float32)
        for h in range(3):
            hidx = 3 * c + h
            img, hf = hidx // 2, hidx % 2
            r0 = hf * 16 - 1
            lo, hi = max(r0, 0), min(r0 + 18, 32)
            fw[h, :, lo - r0:hi - r0, 1:89] = features[img, :, lo:hi, :]
        feat_cores.append(fw.reshape(3, 2, 128, 18, 90))

    shared = {"w1": _round_f32r(w1), "b1": b1, "w2d": _round_f32r(w2dh),
              "w2c": _round_f32r(w2ch),
              "b2d": db2.reshape(DB, 1).astype(np.float32),
              "b2c": cb2.reshape(CTX, 1).astype(np.float32)}
    return [{**shared, "feat": _round_f32r(feat_cores[c])}
            for c in range(N_CORES)]

def _phase1_assemble(results):
    dl = np.zeros((B6, DB, H, W), np.float32)
    ctx = np.zeros((B6, CTX, H, W), np.float32)
    for c in range(N_CORES):
        for h in range(3):
            hidx = 3 * c + h
            img, hf = hidx // 2, hidx % 2
            dl[img, :, hf * 16:(hf + 1) * 16] = results[c]["dl_out"][h]
            ctx[img, :, hf * 16:(hf + 1) * 16] = results[c]["ctx_out"][h]
    return dl, ctx

# ------------------------------------------------------------------ phase 2 --
F16 = mybir.dt.float16

def build_phase2():
    nslot_max = max(len(s) for s in B_SLOTS)
    nc = bacc.Bacc("TRN2", target_bir_lowering=False, debug=False,
                   enable_asserts=True, num_devices=N_CORES)
    pw = nc.dram_tensor("pw", [88, NJ, CB], F32R, kind="ExternalInput").ap()
    wwt = nc.dram_tensor("wwt", [88, 402], F32R, kind="ExternalInput").ap()
    wpq = nc.dram_tensor("wpq", [128, 3, 2, 3, 128], F32R, kind="ExternalInput").ap()
    w2b = nc.dram_tensor("w2b", [128, 128], F32R, kind="ExternalInput").ap()
    wb = nc.dram_tensor("wb", [128, TOT_BSLOTS * 128], F32R, kind="ExternalInput").ap()
    tv = nc.dram_tensor("tv", [128, len(INTERIOR)], F32, kind="ExternalInput").ap()
    bb1 = nc.dram_tensor("bb1", [128, 1], F32, kind="ExternalInput").ap()
    bb2 = nc.dram_tensor("bb2", [128, 1], F32, kind="ExternalInput").ap()
    yout = nc.dram_tensor("yout", [O1, 100, BEV], F32, kind="ExternalOutput").ap()

    II = {i: idx for idx, (i, _) in enumerate(INTERIOR)}

    with tile.TileContext(nc) as tc:
        with tc.tile_pool(name="cp", bufs=1) as cp, \
             tc.tile_pool(name="hpool", bufs=1) as hpool, \
             tc.tile_pool(name="sp", bufs=3) as sp, \
             tc.tile_pool(name="wbp", bufs=8) as wbp, \
             tc.tile_pool(name="ps", bufs=5, space="PSUM") as ps, \
             tc.tile_pool(name="ps1", bufs=3, space="PSUM") as ps1:

            # hrow inputs on the ACT HWDGE queue (first), the rest on SP
            wwtt = cp.tile([88, 402], F32R, tag="wwt")
            nc.scalar.dma_start(wwtt[:], wwt[:])
            pwt = cp.tile([88, NJ, CB], F32R, tag="pw")
            for c3 in range(3):
                nc.scalar.dma_start(pwt[:, c3 * 4:(c3 + 1) * 4, :],
                                    pw[:, c3 * 4:(c3 + 1) * 4, :])
            wpqt = cp.tile([128, 3, 2, 3, 128], F32R, tag="wpq")
            nc.sync.dma_start(wpqt[:], wpq[:])
            w2bt = cp.tile([128, 128], F32R, tag="w2b")
            nc.sync.dma_start(w2bt[:], w2b[:])
            tvt = cp.tile([128, len(INTERIOR)], F32, tag="tv")
            nc.sync.dma_start(tvt[:], tv[:])
            bb1t = cp.tile([128, 1], F32, tag="bb1")
            nc.sync.dma_start(bb1t[:], bb1[:])
            bb2t = cp.tile([128, 1], F32, tag="bb2")
            nc.sync.dma_start(bb2t[:], bb2[:])

            hr = [hpool.tile([128, NJ, 402], F32R, tag="hr0", name="hr0"),
                  hpool.tile([64, NJ, 402], F32R, tag="hr1", name="hr1")]
            for jr in range(NJ):
                for kt in range(2):
                    m = 128 if kt == 0 else 64
                    ph = ps.tile([128, 402], F32, tag="acc", name="ph")
                    nc.tensor.matmul(ph[0:m, :],
                                     pwt[:, jr, kt * 128:kt * 128 + m],
                                     wwtt[:], start=True, stop=True)
                    nc.scalar.copy(hr[kt][:, jr, :], ph[0:m, :])
            dh = [hpool.tile([128, NJ - 1, 402], F32R, tag="dh0", name="dh0"),
                  hpool.tile([64, NJ - 1, 402], F32R, tag="dh1", name="dh1")]
            for jr in range(NJ - 1):
                for kt in range(2):
                    nc.vector.tensor_tensor(dh[kt][:, jr, :], hr[kt][:, jr + 1, :],
                                            hr[kt][:, jr, :], ALU.subtract)

            def finish_pair(i, src):
                gt = sp.tile([128, 400], F32R, tag="gt", name="gt")
                nc.scalar.activation(gt[:], src, AF.Gelu, bias=bb1t[:])
                p1_ = ps1.tile([128, 400], F32, tag="acc1", name="p1_")
                nc.tensor.matmul(p1_[:], w2bt[:], gt[:], start=True, stop=True)
                ot = sp.tile([128, 400], F32, tag="ot", name="ot")
                nc.vector.tensor_scalar(ot[:], p1_[:], bb2t[:], None, ALU.add)
                nc.scalar.dma_start(
                    yout[:, 2 * i:2 * i + 2, :].rearrange("o g x -> g o x"), ot[:])

            def do_boundary(bi):
                i, js = BOUNDARY[bi]
                nsl = len(B_SLOTS[bi])
                wbt = wbp.tile([128, nslot_max * 128], F32R, tag="wbt", name="wbt")
                nc.sync.dma_start(wbt[:, 0:nsl * 128],
                                  wb[:, B_OFF[bi] * 128:B_OFF[bi + 1] * 128])
                psB = ps.tile([128, 400], F32, tag="acc", name="psB")
                for si, (kt, t, j) in enumerate(B_SLOTS[bi]):
                    m = 128 if kt == 0 else 64
                    nc.tensor.matmul(psB[:], wbt[0:m, si * 128:si * 128 + 128],
                                     hr[kt][:, j + 1, t:t + 400],
                                     start=(si == 0), stop=(si == nsl - 1))
                finish_pair(i, psB[:])

            bq = list(range(len(BOUNDARY)))

            def do_cell_mms(a):
                ar = a + 1
                psP = ps.tile([128, 400], F32, tag="acc", name="psP")
                n = 0
                for sidx, jr in ((0, ar), (1, ar + 1)):
                    for t in range(3):
                        for kt in range(2):
                            m = 128 if kt == 0 else 64
                            nc.tensor.matmul(psP[:], wpqt[0:m, sidx, kt, t, :],
                                             hr[kt][:, jr, t:t + 400],
                                             start=(n == 0), stop=(n == 11))
                            n += 1
                psQ = ps.tile([128, 400], F32, tag="acc", name="psQ")
                n = 0
                for t in range(3):
                    for kt in range(2):
                        m = 128 if kt == 0 else 64
                        nc.tensor.matmul(psQ[:], wpqt[0:m, 2, kt, t, :],
                                         dh[kt][:, ar, t:t + 400],
                                         start=(n == 0), stop=(n == 5))
                        n += 1
                Pt = sp.tile([128, 400], F32R, tag="Pt", name="Pt")
                nc.scalar.copy(Pt[:], psP[:])
                Qt = sp.tile([128, 400], F32R, tag="Qt", name="Qt")
                nc.vector.tensor_copy(Qt[:], psQ[:])
                return Pt, Qt

            def do_cell_pairs(a, Pt, Qt):
                for i in PAIRS_OF_CELL[a]:
                    idx = II[i]
                    yt = sp.tile([128, 400], F32R, tag="yt", name="yt")
                    nc.vector.tensor_scalar(yt[:], Qt[:], tvt[:, idx:idx + 1],
                                            None, ALU.mult)
                    nc.vector.tensor_tensor(yt[:], yt[:], Pt[:], ALU.add)
                    finish_pair(i, yt[:])

            # one-stage software pipeline: next cell's MMs before this cell's
            # row-finishes, boundary groups as PE filler in between
            prev = None
            for ci, a in enumerate(CELLS):
                pq = do_cell_mms(a)
                if prev is not None:
                    if ci >= 2 and bq:
                        do_boundary(bq.pop(0))
                    do_cell_pairs(*prev)
                prev = (a, *pq)
            if prev is not None:
                do_cell_pairs(prev[0], prev[1], prev[2])
            while bq:
                do_boundary(bq.pop(0))
    nc.compile()
    return nc

def _coef(rr_abs, j_abs):
    if rr_abs < 0 or rr_abs >= BEV:
        return 0.0
    s = (rr_abs + 0.5) * 0.08 - 0.5
    a = int(np.floor(s)); t = s - a
    return (1 - t) if j_abs == a else (t if j_abs == a + 1 else 0.0)

def _phase2_inputs(pooled, bw1, bb1, bw2, bb2):
    Ww = _interp_matrix(W, BEV)
    wwt = np.zeros((88, 402), np.float32)
    wwt[:, 1:401] = Ww.T

    Wsum = bw1.sum(axis=2)                    # (O1, CB, 3)
    Wdiff = bw1[:, :, 2] - bw1[:, :, 0]
    Wsets = [Wsum - 0.08 * Wdiff, 0.08 * Wdiff, Wsum]
    wpq = np.zeros((3, 2, 3, 128, 128), np.float32)
    for s_ in range(3):
        for kt in range(2):
            m = 128 if kt == 0 else 64
            for t in range(3):
                blk = Wsets[s_][:, kt * 128:kt * 128 + m, t]   # (O1, m)
                for g in range(2):
                    wpq[s_, kt, t, 0:m, g * 64:g * 64 + 64] = blk.T
    w2b = np.zeros((128, 128), np.float32)
    for g in range(2):
        w2b[g * 64:g * 64 + 64, g * 64:g * 64 + 64] = bw2[:, :, 0, 0].T
    tv = np.zeros((128, len(INTERIOR)), np.float32)
    for idx, (i, a) in enumerate(INTERIOR):
        for g in range(2):
            tv[g * 64:(g + 1) * 64, idx] = (2 * i + g + 0.5) * 0.08 - 0.5 - a
    bb1d = np.tile(bb1, 2).reshape(128, 1).astype(np.float32)
    bb2d = np.tile(bb2, 2).reshape(128, 1).astype(np.float32)

    shared = {"wwt": _round_f32r(wwt),
              "wpq": _round_f32r(wpq.transpose(3, 0, 1, 2, 4).copy()),
              "w2b": _round_f32r(w2b), "tv": tv, "bb1": bb1d, "bb2": bb2d}

    in_maps = []
    for c in range(N_CORES):
        bat, blk = c // 4, c % 4
        pwin = np.zeros((88, NJ, CB), np.float32)
        for jr in range(NJ):
            j = int(np.clip(8 * blk - 1 + jr, 0, H - 1))
            pwin[:, jr, :] = pooled[bat, :, j, :].T
        wbv = np.zeros((128, TOT_BSLOTS * 128), np.float32)
        for bi, (i, js) in enumerate(BOUNDARY):
            for si, (kt, t, j) in enumerate(B_SLOTS[bi]):
                gs = B_OFF[bi] + si
                m = 128 if kt == 0 else 64
                for g in range(2):
                    W_ = np.zeros((O1, m), np.float32)
                    for d in range(3):
                        cf = _coef(100 * blk + 2 * i + g + d - 1, 8 * blk + j)
                        if cf != 0.0:
                            W_ += cf * bw1[:, kt * 128:kt * 128 + m, d, t]
                    wbv[0:m, gs * 128 + g * 64:gs * 128 + g * 64 + 64] = W_.T
        in_maps.append({**shared, "pw": _round_f32r(pwin),
                        "wb": _round_f32r(wbv)})
    return in_maps

def _phase2_assemble(results):
    out = np.zeros((2, O1, BEV, BEV), np.float32)
    for c in range(N_CORES):
        bat, blk = c // 4, c % 4
        out[bat, :, 100 * blk:100 * blk + 100, :] = results[c]["yout"]
    return out

# ------------------------------------------------------------------- cache --
_NC_CACHE = {}

def _get_nc(name, builder):
    if name not in _NC_CACHE:
        _NC_CACHE[name] = builder()
    return _NC_CACHE[name]

def kernel(**inputs):
    inputs = {k: np.asarray(v) for k, v in inputs.items()}
    p1_keys = ['features', 'dw1', 'db1', 'dgamma', 'dbeta', 'dmean', 'dvar',
               'dw2', 'db2', 'cw1', 'cb1', 'cgamma', 'cbeta', 'cmean', 'cvar',
               'cw2', 'cb2']
    nc1 = _get_nc("p1", build_phase1)
    in_maps = _phase1_inputs(*[inputs[k] for k in p1_keys])
    res1 = bass_utils.run_bass_kernel_spmd(nc1, in_maps,
                                           core_ids=list(range(N_CORES)))
    dl, ctx = _phase1_assemble(res1.results)
    pooled = ctx.reshape(2, 6 * CTX, H, W)

    nc2 = _get_nc("p2", build_phase2)
    in_maps2 = _phase2_inputs(pooled, inputs['bw1'], inputs['bb1'],
                              inputs['bw2'], inputs['bb2'])
    res2 = bass_utils.run_bass_kernel_spmd(nc2, in_maps2,
                                           core_ids=list(range(N_CORES)))
    out = _phase2_assemble(res2.results)
    return out, dl


# revision 29
# speedup vs baseline: 1.0031x; 1.0031x over previous
"""Trainium2 Bass kernel for nn_DepthAwareProjector.

Two SPMD launches on 8 NeuronCores:
  Phase 1 (per-camera nets): 24 half-images (12 cams x 2 halves) -> 3 per core.
    depth branch: conv3x3(256->256)+BN+ReLU -> conv1x1(256->41) = depth_logits
    ctx branch:   conv3x3(256->256)+BN+ReLU -> conv1x1(256->32) = context
    (softmax-sum over depth bins == 1, so pooled == context)
  Phase 2 (bev): bilinear 32x88 -> 400x400 fused with conv3x3(192->64),
    gelu, conv1x1(64->64). Sharded (batch 2 x 4 row-blocks of 100) -- blocks
    align with the period-25 interp phase pattern so all cores run one program.

All matmuls in float32r (full PE rate at N>=256, ~1.5e-4 rel err).
"""
import sys
if '/opt/trn_rl_repo' not in sys.path:
    sys.path.insert(0, '/opt/trn_rl_repo')

import numpy as np
import concourse.bass as bass
import concourse.tile as tile
from concourse import bacc, mybir
from concourse import bass_utils

F32 = mybir.dt.float32
F32R = mybir.dt.float32r
AF = mybir.ActivationFunctionType
ALU = mybir.AluOpType

N_CORES = 8
B6, CIN, H, W = 12, 256, 32, 88
DB, CTX = 41, 32
BEV, O1, CB = 400, 64, 192   # bev dim, compressor out ch, compressor in ch
NJ = 12                      # pooled-window rows per core (jrel 0..11)

# ---------------------------------------------------------------- schedule --
def _s_of(rr):
    return (rr + 0.5) * 0.08 - 0.5

def _cell_of(rr):
    return int(np.floor(_s_of(rr)))

def _schedule():
    """Row-pair classification for a 100-row block (block-independent)."""
    interior, boundary = [], []
    for i in range(50):
        cells = {_cell_of(2 * i + q + d - 1) for q in (0, 1) for d in range(3)}
        if len(cells) == 1 and i not in (0, 49):
            interior.append((i, min(cells)))
        else:
            js = sorted({c for c in cells} | {c + 1 for c in cells})
            boundary.append((i, js))
    return interior, boundary

INTERIOR, BOUNDARY = _schedule()
CELLS = sorted({a for _, a in INTERIOR})
PAIRS_OF_CELL = {a: [i for i, aa in INTERIOR if aa == a] for a in CELLS}
# boundary slot layout: per pair, slots = [(kt, t, j) for j in js for t in 3 for kt in 2]
B_SLOTS = [[(kt, t, j) for j in js for t in range(3) for kt in range(2)]
           for _, js in BOUNDARY]
B_OFF = np.cumsum([0] + [len(s) for s in B_SLOTS]).tolist()
TOT_BSLOTS = B_OFF[-1]

def _interp_matrix(n_src, n_dst):
    M = np.zeros((n_dst, n_src), np.float32)
    for d in range(n_dst):
        s = (d + 0.5) * n_src / n_dst - 0.5
        s0 = int(np.floor(s)); t = s - s0
        M[d, np.clip(s0, 0, n_src - 1)] += 1 - t
        M[d, np.clip(s0 + 1, 0, n_src - 1)] += t
    return M

def _round_f32r(a):
    """Round to fp32r (11-bit mantissa, RNE) so HWDGE DMAs need no cast."""
    b = np.ascontiguousarray(a, np.float32).view(np.uint32)
    drop = 12
    half = np.uint32(1 << (drop - 1))
    mask = np.uint32((1 << drop) - 1)
    low = b & mask
    r = b & ~mask
    add = (low > half) | ((low == half) & (((b >> drop) & 1) == 1))
    return (r + (add.astype(np.uint32) << drop)).view(np.float32)

# ------------------------------------------------------------------ phase 1 --
def build_phase1():
    nc = bacc.Bacc("TRN2", target_bir_lowering=False, debug=False,
                   enable_asserts=True, num_devices=N_CORES)
    feat = nc.dram_tensor("feat", [3, 2, 128, 18, 90], F32R, kind="ExternalInput").ap()
    w1 = nc.dram_tensor("w1", [2, 2, 128, 9 * 256], F32R, kind="ExternalInput").ap()
    b1 = nc.dram_tensor("b1", [128, 4], F32, kind="ExternalInput").ap()
    w2d = nc.dram_tensor("w2d", [2, 128, DB], F32R, kind="ExternalInput").ap()
    w2c = nc.dram_tensor("w2c", [2, 128, CTX], F32R, kind="ExternalInput").ap()
    b2d = nc.dram_tensor("b2d", [DB, 1], F32, kind="ExternalInput").ap()
    b2c = nc.dram_tensor("b2c", [CTX, 1], F32, kind="ExternalInput").ap()
    dl_out = nc.dram_tensor("dl_out", [3, DB, 16, 88], F32, kind="ExternalOutput").ap()
    ctx_out = nc.dram_tensor("ctx_out", [3, CTX, 16, 88], F32, kind="ExternalOutput").ap()

    with tile.TileContext(nc) as tc:
        with tc.tile_pool(name="wp", bufs=1) as wp, \
             tc.tile_pool(name="fp", bufs=2) as fp, \
             tc.tile_pool(name="hp", bufs=2) as hp, \
             tc.tile_pool(name="op", bufs=2) as op, \
             tc.tile_pool(name="ps", bufs=6, space="PSUM") as ps, \
             tc.tile_pool(name="ps2", bufs=2, space="PSUM") as ps2:

            # feat for half 0 first so the first matmul group starts early
            ft0 = []
            for kt in range(2):
                t_ = fp.tile([128, 18, 90], F32R, tag=f"f{kt}", name=f"f{kt}")
                nc.sync.dma_start(t_[:], feat[0, kt, :, :, :])
                ft0.append(t_)
            w1ts = {}
            for br in range(2):
                for kt in range(2):
                    t_ = wp.tile([128, 9 * 256], F32R, tag=f"w1{br}{kt}",
                                 name=f"w1{br}{kt}")
                    for c3 in range(3):
                        nc.scalar.dma_start(t_[:, c3 * 768:(c3 + 1) * 768],
                                            w1[br, kt, :, c3 * 768:(c3 + 1) * 768])
                    w1ts[br, kt] = t_
            w2dt = wp.tile([128, 2, DB], F32R, tag="w2d")
            w2ct = wp.tile([128, 2, CTX], F32R, tag="w2c")
            for kt in range(2):
                nc.sync.dma_start(w2dt[:, kt, :], w2d[kt, :, :])
                nc.sync.dma_start(w2ct[:, kt, :], w2c[kt, :, :])
            b1t = wp.tile([128, 4], F32, tag="b1")
            nc.sync.dma_start(b1t[:], b1[:])
            b2dt = wp.tile([DB, 1], F32, tag="b2d")
            nc.sync.dma_start(b2dt[:], b2d[:])
            b2ct = wp.tile([CTX, 1], F32, tag="b2c")
            nc.sync.dma_start(b2ct[:], b2c[:])

            for half in range(3):
                if half == 0:
                    ft = ft0
                else:
                    ft = []
                    for kt in range(2):
                        t_ = fp.tile([128, 18, 90], F32R, tag=f"f{kt}",
                                     name=f"f{kt}")
                        nc.sync.dma_start(t_[:], feat[half, kt, :, :, :])
                        ft.append(t_)
                for br in range(2):
                    ht = [hp.tile([128, 16, 88], F32R, tag=f"h{br}{m}", name=f"h{br}{m}")
                          for m in range(2)]
                    if br == 0:
                        w2t, b2t, no, dst = w2dt, b2dt, DB, dl_out
                    else:
                        w2t, b2t, no, dst = w2ct, b2ct, CTX, ctx_out
                    st = op.tile([no, 16, 88], F32, tag=f"o{br}", name=f"o{br}")
                    for chunk in range(4):
                        for m in range(2):
                            pst = ps.tile([128, 4, 88], F32, tag="acc")
                            n = 0
                            for kt in range(2):
                                for dy in range(3):
                                    for dx in range(3):
                                        nc.tensor.matmul(
                                            pst[:],
                                            w1ts[br, kt][:,
                                                (dy * 3 + dx) * 256 + m * 128:
                                                (dy * 3 + dx) * 256 + m * 128 + 128],
                                            ft[kt][:, chunk * 4 + dy:chunk * 4 + dy + 4,
                                                   dx:dx + 88],
                                            start=(n == 0), stop=(n == 17))
                                        n += 1
                            nc.scalar.activation(
                                ht[m][:, chunk * 4:chunk * 4 + 4, :], pst[:],
                                AF.Relu, bias=b1t[:, br * 2 + m:br * 2 + m + 1])
                        p2 = ps2.tile([128, 4, 88], F32, tag="acc2")
                        for kt in range(2):
                            nc.tensor.matmul(
                                p2[0:no, :, :], w2t[:, kt, :],
                                ht[kt][:, chunk * 4:chunk * 4 + 4, :],
                                start=(kt == 0), stop=(kt == 1))
                        nc.scalar.activation(
                            st[:, chunk * 4:chunk * 4 + 4, :], p2[0:no, :, :],
                            AF.Identity, bias=b2t[:])
                    nc.sync.dma_start(dst[half], st[:])
    nc.compile()
    return nc

# ---------------------------------------------------------------- host prep --
def _fold_bn(w, b, gamma, beta, mean, var, eps=1e-5):
    sc = gamma / np.sqrt(var + eps)
    return (w * sc[:, None, None, None]).astype(np.float32), \
           (b * sc + (beta - mean * sc)).astype(np.float32)

def _phase1_inputs(features, dw1, db1, dgamma, dbeta, dmean, dvar, dw2, db2,
                   cw1, cb1, cgamma, cbeta, cmean, cvar, cw2, cb2):
    dw1f, db1f = _fold_bn(dw1, db1, dgamma, dbeta, dmean, dvar)
    cw1f, cb1f = _fold_bn(cw1, cb1, cgamma, cbeta, cmean, cvar)

    w1 = np.zeros((2, 2, 128, 9 * 256), np.float32)
    for br, wf in ((0, dw1f), (1, cw1f)):
        # lhsT[c_in, o] per tap; layout [kt, c, tap*256 + o]
        t_ = wf.transpose(1, 2, 3, 0).reshape(256, 9, 256)  # [c_in, tap, o]
        for kt in range(2):
            w1[br, kt] = t_[kt * 128:(kt + 1) * 128].reshape(128, 9 * 256)
    b1 = np.zeros((128, 4), np.float32)
    for br, bf in ((0, db1f), (1, cb1f)):
        for m in range(2):
            b1[:, br * 2 + m] = bf[m * 128:(m + 1) * 128]
    w2dh = dw2[:, :, 0, 0].T.reshape(2, 128, DB).astype(np.float32)
    w2ch = cw2[:, :, 0, 0].T.reshape(2, 128, CTX).astype(np.float32)

    feat_cores = []
    for c in range(N_CORES):
        fw = np.zeros((3, 256, 18, 90), np.float32)
        for h in range(3):
            hidx = 3 * c + h
            img, hf = hidx // 2, hidx % 2
            r0 = hf * 16 - 1
            lo, hi = max(r0, 0), min(r0 + 18, 32)
            fw[h, :, lo - r0:hi - r0, 1:89] = features[img, :, lo:hi, :]
        feat_cores.append(fw.reshape(3, 2, 128, 18, 90))

    shared = {"w1": _round_f32r(w1), "b1": b1, "w2d": _round_f32r(w2dh),
              "w2c": _round_f32r(w2ch),
              "b2d": db2.reshape(DB, 1).astype(np.float32),
              "b2c": cb2.reshape(CTX, 1).astype(np.float32)}
    return [{**shared, "feat": _round_f32r(feat_cores[c])}
            for c in range(N_CORES)]

def _phase1_assemble(results):
    dl = np.zeros((B6, DB, H, W), np.float32)
    ctx = np.zeros((B6, CTX, H, W), np.float32)
    for c in range(N_CORES):
        for h in range(3):
            hidx = 3 * c + h
            img, hf = hidx // 2, hidx % 2
            dl[img, :, hf * 16:(hf + 1) * 16] = results[c]["dl_out"][h]
            ctx[img, :, hf * 16:(hf + 1) * 16] = results[c]["ctx_out"][h]
    return dl, ctx

# ------------------------------------------------------------------ phase 2 --
F16 = mybir.dt.float16

def build_phase2():
    nslot_max = max(len(s) for s in B_SLOTS)
    nc = bacc.Bacc("TRN2", target_bir_lowering=False, debug=False,
                   enable_asserts=True, num_devices=N_CORES)
    pw = nc.dram_tensor("pw", [88, NJ, CB], F32R, kind="ExternalInput").ap()
    wwt = nc.dram_tensor("wwt", [88, 402], F32R, kind="ExternalInput").ap()
    wpq = nc.dram_tensor("wpq", [128, 3, 2, 3, 128], F32R, kind="ExternalInput").ap()
    w2b = nc.dram_tensor("w2b", [128, 128], F32R, kind="ExternalInput").ap()
    wb = nc.dram_tensor("wb", [128, TOT_BSLOTS * 128], F32R, kind="ExternalInput").ap()
    tv = nc.dram_tensor("tv", [128, len(INTERIOR)], F32, kind="ExternalInput").ap()
    bb1 = nc.dram_tensor("bb1", [128, 1], F32, kind="ExternalInput").ap()
    bb2 = nc.dram_tensor("bb2", [128, 1], F32, kind="ExternalInput").ap()
    yout = nc.dram_tensor("yout", [O1, 100, BEV], F32, kind="ExternalOutput").ap()

    II = {i: idx for idx, (i, _) in enumerate(INTERIOR)}

    with tile.TileContext(nc) as tc:
        with tc.tile_pool(name="cp", bufs=1) as cp, \
             tc.tile_pool(name="hpool", bufs=1) as hpool, \
             tc.tile_pool(name="sp", bufs=3) as sp, \
             tc.tile_pool(name="wbp", bufs=8) as wbp, \
             tc.tile_pool(name="ps", bufs=5, space="PSUM") as ps, \
             tc.tile_pool(name="ps1", bufs=3, space="PSUM") as ps1:

            # hrow inputs on the ACT HWDGE queue (first), the rest on SP
            wwtt = cp.tile([88, 402], F32R, tag="wwt")
            nc.scalar.dma_start(wwtt[:], wwt[:])
            pwt = cp.tile([88, NJ, CB], F32R, tag="pw")
            for c3 in range(3):
                nc.scalar.dma_start(pwt[:, c3 * 4:(c3 + 1) * 4, :],
                                    pw[:, c3 * 4:(c3 + 1) * 4, :])
            wpqt = cp.tile([128, 3, 2, 3, 128], F32R, tag="wpq")
            nc.sync.dma_start(wpqt[:], wpq[:])
            w2bt = cp.tile([128, 128], F32R, tag="w2b")
            nc.sync.dma_start(w2bt[:], w2b[:])
            tvt = cp.tile([128, len(INTERIOR)], F32, tag="tv")
            nc.sync.dma_start(tvt[:], tv[:])
            bb1t = cp.tile([128, 1], F32, tag="bb1")
            nc.sync.dma_start(bb1t[:], bb1[:])
            bb2t = cp.tile([128, 1], F32, tag="bb2")
            nc.sync.dma_start(bb2t[:], bb2[:])

            hr = [hpool.tile([128, NJ, 402], F32R, tag="hr0", name="hr0"),
                  hpool.tile([64, NJ, 402], F32R, tag="hr1", name="hr1")]
            for jr in range(NJ):
                for kt in range(2):
                    m = 128 if kt == 0 else 64
                    ph = ps.tile([128, 402], F32, tag="acc", name="ph")
                    nc.tensor.matmul(ph[0:m, :],
                                     pwt[:, jr, kt * 128:kt * 128 + m],
                                     wwtt[:], start=True, stop=True)
                    nc.scalar.copy(hr[kt][:, jr, :], ph[0:m, :])
            dh = [hpool.tile([128, NJ - 1, 402], F32R, tag="dh0", name="dh0"),
                  hpool.tile([64, NJ - 1, 402], F32R, tag="dh1", name="dh1")]
            for jr in range(NJ - 1):
                for kt in range(2):
                    nc.vector.tensor_tensor(dh[kt][:, jr, :], hr[kt][:, jr + 1, :],
                                            hr[kt][:, jr, :], ALU.subtract)

            def finish_pair(i, src):
                gt = sp.tile([128, 400], F32R, tag="gt", name="gt")
                nc.scalar.activation(gt[:], src, AF.Gelu, bias=bb1t[:])
                p1_ = ps1.tile([128, 400], F32, tag="acc1", name="p1_")
                nc.tensor.matmul(p1_[:], w2bt[:], gt[:], start=True, stop=True)
                ot = sp.tile([128, 400], F32, tag="ot", name="ot")
                nc.vector.tensor_scalar(ot[:], p1_[:], bb2t[:], None, ALU.add)
                nc.scalar.dma_start(
                    yout[:, 2 * i:2 * i + 2, :].rearrange("o g x -> g o x"), ot[:])

            def do_boundary(bi):
                i, js = BOUNDARY[bi]
                nsl = len(B_SLOTS[bi])
                wbt = wbp.tile([128, nslot_max * 128], F32R, tag="wbt", name="wbt")
                nc.sync.dma_start(wbt[:, 0:nsl * 128],
                                  wb[:, B_OFF[bi] * 128:B_OFF[bi + 1] * 128])
                psB = ps.tile([128, 400], F32, tag="acc", name="psB")
                for si, (kt, t, j) in enumerate(B_SLOTS[bi]):
                    m = 128 if kt == 0 else 64
                    nc.tensor.matmul(psB[:], wbt[0:m, si * 128:si * 128 + 128],
                                     hr[kt][:, j + 1, t:t + 400],
                                     start=(si == 0), stop=(si == nsl - 1))
                finish_pair(i, psB[:])

            bq = list(range(len(BOUNDARY)))

            def do_cell_mms(a):
                ar = a + 1
                psP = ps.tile([128, 400], F32, tag="acc", name="psP")
                n = 0
                for sidx, jr in ((0, ar), (1, ar + 1)):
                    for t in range(3):
                        for kt in range(2):
                            m = 128 if kt == 0 else 64
                            nc.tensor.matmul(psP[:], wpqt[0:m, sidx, kt, t, :],
                                             hr[kt][:, jr, t:t + 400],
                                             start=(n == 0), stop=(n == 11))
                            n += 1
                psQ = ps.tile([128, 400], F32, tag="acc", name="psQ")
                n = 0
                for t in range(3):
                    for kt in range(2):
                        m = 128 if kt == 0 else 64
                        nc.tensor.matmul(psQ[:], wpqt[0:m, 2, kt, t, :],
                                         dh[kt][:, ar, t:t + 400],
                                         start=(n == 0), stop=(n == 5))
                        n += 1
                Pt = sp.tile([128, 400], F32R, tag="Pt", name="Pt")
                nc.scalar.copy(Pt[:], psP[:])
                Qt = sp.tile([128, 400], F32R, tag="Qt", name="Qt")
                nc.vector.tensor_copy(Qt[:], psQ[:])
                return Pt, Qt

            def do_cell_pairs(a, Pt, Qt):
                for i in PAIRS_OF_CELL[a]:
                    idx = II[i]
                    yt = sp.tile([128, 400], F32R, tag="yt", name="yt")
                    nc.vector.tensor_scalar(yt[:], Qt[:], tvt[:, idx:idx + 1],
                                            None, ALU.mult)
                    nc.vector.tensor_tensor(yt[:], yt[:], Pt[:], ALU.add)
                    finish_pair(i, yt[:])

            # one-stage software pipeline: next cell's MMs before this cell's
            # row-finishes, boundary groups as PE filler in between
            prev = None
            for ci, a in enumerate(CELLS):
                pq = do_cell_mms(a)
                if prev is not None:
                    if ci >= 1 and bq:
                        do_boundary(bq.pop(0))
                    if ci >= 3 and bq:
                        do_boundary(bq.pop(0))
                    do_cell_pairs(*prev)
                prev = (a, *pq)
            if prev is not None:
                do_cell_pairs(prev[0], prev[1], prev[2])
            while bq:
                do_boundary(bq.pop(0))
    nc.compile()
    return nc

def _coef(rr_abs, j_abs):
    if rr_abs < 0 or rr_abs >= BEV:
        return 0.0
    s = (rr_abs + 0.5) * 0.08 - 0.5
    a = int(np.floor(s)); t = s - a
    return (1 - t) if j_abs == a else (t if j_abs == a + 1 else 0.0)

def _phase2_inputs(pooled, bw1, bb1, bw2, bb2):
    Ww = _interp_matrix(W, BEV)
    wwt = np.zeros((88, 402), np.float32)
    wwt[:, 1:401] = Ww.T

    Wsum = bw1.sum(axis=2)                    # (O1, CB, 3)
    Wdiff = bw1[:, :, 2] - bw1[:, :, 0]
    Wsets = [Wsum - 0.08 * Wdiff, 0.08 * Wdiff, Wsum]
    wpq = np.zeros((3, 2, 3, 128, 128), np.float32)
    for s_ in range(3):
        for kt in range(2):
            m = 128 if kt == 0 else 64
            for t in range(3):
                blk = Wsets[s_][:, kt * 128:kt * 128 + m, t]   # (O1, m)
                for g in range(2):
                    wpq[s_, kt, t, 0:m, g * 64:g * 64 + 64] = blk.T
    w2b = np.zeros((128, 128), np.float32)
    for g in range(2):
        w2b[g * 64:g * 64 + 64, g * 64:g * 64 + 64] = bw2[:, :, 0, 0].T
    tv = np.zeros((128, len(INTERIOR)), np.float32)
    for idx, (i, a) in enumerate(INTERIOR):
        for g in range(2):
            tv[g * 64:(g + 1) * 64, idx] = (2 * i + g + 0.5) * 0.08 - 0.5 - a
    bb1d = np.tile(bb1, 2).reshape(128, 1).astype(np.float32)
    bb2d = np.tile(bb2, 2).reshape(128, 1).astype(np.float32)

    shared = {"wwt": _round_f32r(wwt),
              "wpq": _round_f32r(wpq.transpose(3, 0, 1, 2, 4).copy()),
              "w2b": _round_f32r(w2b), "tv": tv, "bb1": bb1d, "bb2": bb2d}

    in_maps = []
    for c in range(N_CORES):
        bat, blk = c // 4, c % 4
        pwin = np.zeros((88, NJ, CB), np.float32)
        for jr in range(NJ):
            j = int(np.clip(8 * blk - 1 + jr, 0, H - 1))
            pwin[:, jr, :] = pooled[bat, :, j, :].T
        wbv = np.zeros((128, TOT_BSLOTS * 128), np.float32)
        for bi, (i, js) in enumerate(BOUNDARY):
            for si, (kt, t, j) in enumerate(B_SLOTS[bi]):
                gs = B_OFF[bi] + si
                m = 128 if kt == 0 else 64
                for g in range(2):
                    W_ = np.zeros((O1, m), np.float32)
                    for d in range(3):
                        cf = _coef(100 * blk + 2 * i + g + d - 1, 8 * blk + j)
                        if cf != 0.0:
                            W_ += cf * bw1[:, kt * 128:kt * 128 + m, d, t]
                    wbv[0:m, gs * 128 + g * 64:gs * 128 + g * 64 + 64] = W_.T
        in_maps.append({**shared, "pw": _round_f32r(pwin),
                        "wb": _round_f32r(wbv)})
    return in_maps

def _phase2_assemble(results):
    out = np.zeros((2, O1, BEV, BEV), np.float32)
    for c in range(N_CORES):
        bat, blk = c // 4, c % 4
        out[bat, :, 100 * blk:100 * blk + 100, :] = results[c]["yout"]
    return out

# ------------------------------------------------------------------- cache --
_NC_CACHE = {}

def _get_nc(name, builder):
    if name not in _NC_CACHE:
        _NC_CACHE[name] = builder()
    return _NC_CACHE[name]

def kernel(**inputs):
    inputs = {k: np.asarray(v) for k, v in inputs.items()}
    p1_keys = ['features', 'dw1', 'db1', 'dgamma', 'dbeta', 'dmean', 'dvar',
               'dw2', 'db2', 'cw1', 'cb1', 'cgamma', 'cbeta', 'cmean', 'cvar',
               'cw2', 'cb2']
    nc1 = _get_nc("p1", build_phase1)
    in_maps = _phase1_inputs(*[inputs[k] for k in p1_keys])
    res1 = bass_utils.run_bass_kernel_spmd(nc1, in_maps,
                                           core_ids=list(range(N_CORES)))
    dl, ctx = _phase1_assemble(res1.results)
    pooled = ctx.reshape(2, 6 * CTX, H, W)

    nc2 = _get_nc("p2", build_phase2)
    in_maps2 = _phase2_inputs(pooled, inputs['bw1'], inputs['bb1'],
                              inputs['bw2'], inputs['bb2'])
    res2 = bass_utils.run_bass_kernel_spmd(nc2, in_maps2,
                                           core_ids=list(range(N_CORES)))
    out = _phase2_assemble(res2.results)
    return out, dl


# revision 30
# speedup vs baseline: 1.0137x; 1.0106x over previous
"""Trainium2 Bass kernel for nn_DepthAwareProjector.

Two SPMD launches on 8 NeuronCores:
  Phase 1 (per-camera nets): 24 half-images (12 cams x 2 halves) -> 3 per core.
    depth branch: conv3x3(256->256)+BN+ReLU -> conv1x1(256->41) = depth_logits
    ctx branch:   conv3x3(256->256)+BN+ReLU -> conv1x1(256->32) = context
    (softmax-sum over depth bins == 1, so pooled == context)
  Phase 2 (bev): bilinear 32x88 -> 400x400 fused with conv3x3(192->64),
    gelu, conv1x1(64->64). Sharded (batch 2 x 4 row-blocks of 100) -- blocks
    align with the period-25 interp phase pattern so all cores run one program.

All matmuls in float32r (full PE rate at N>=256, ~1.5e-4 rel err).
"""
import sys
if '/opt/trn_rl_repo' not in sys.path:
    sys.path.insert(0, '/opt/trn_rl_repo')

import numpy as np
import concourse.bass as bass
import concourse.tile as tile
from concourse import bacc, mybir
from concourse import bass_utils

F32 = mybir.dt.float32
F32R = mybir.dt.float32r
AF = mybir.ActivationFunctionType
ALU = mybir.AluOpType

N_CORES = 8
B6, CIN, H, W = 12, 256, 32, 88
DB, CTX = 41, 32
BEV, O1, CB = 400, 64, 192   # bev dim, compressor out ch, compressor in ch
NJ = 12                      # pooled-window rows per core (jrel 0..11)

# ---------------------------------------------------------------- schedule --
def _s_of(rr):
    return (rr + 0.5) * 0.08 - 0.5

def _cell_of(rr):
    return int(np.floor(_s_of(rr)))

def _schedule():
    """Row-pair classification for a 100-row block (block-independent)."""
    interior, boundary = [], []
    for i in range(50):
        cells = {_cell_of(2 * i + q + d - 1) for q in (0, 1) for d in range(3)}
        if len(cells) == 1 and i not in (0, 49):
            interior.append((i, min(cells)))
        else:
            js = sorted({c for c in cells} | {c + 1 for c in cells})
            boundary.append((i, js))
    return interior, boundary

INTERIOR, BOUNDARY = _schedule()
CELLS = sorted({a for _, a in INTERIOR})
PAIRS_OF_CELL = {a: [i for i, aa in INTERIOR if aa == a] for a in CELLS}
# boundary slot layout: per pair, slots = [(kt, t, j) for j in js for t in 3 for kt in 2]
B_SLOTS = [[(kt, t, j) for j in js for t in range(3) for kt in range(2)]
           for _, js in BOUNDARY]
B_OFF = np.cumsum([0] + [len(s) for s in B_SLOTS]).tolist()
TOT_BSLOTS = B_OFF[-1]

def _interp_matrix(n_src, n_dst):
    M = np.zeros((n_dst, n_src), np.float32)
    for d in range(n_dst):
        s = (d + 0.5) * n_src / n_dst - 0.5
        s0 = int(np.floor(s)); t = s - s0
        M[d, np.clip(s0, 0, n_src - 1)] += 1 - t
        M[d, np.clip(s0 + 1, 0, n_src - 1)] += t
    return M

def _round_f32r(a):
    """Round to fp32r (11-bit mantissa, RNE) so HWDGE DMAs need no cast."""
    b = np.ascontiguousarray(a, np.float32).view(np.uint32)
    drop = 12
    half = np.uint32(1 << (drop - 1))
    mask = np.uint32((1 << drop) - 1)
    low = b & mask
    r = b & ~mask
    add = (low > half) | ((low == half) & (((b >> drop) & 1) == 1))
    return (r + (add.astype(np.uint32) << drop)).view(np.float32)

# ------------------------------------------------------------------ phase 1 --
def build_phase1():
    nc = bacc.Bacc("TRN2", target_bir_lowering=False, debug=False,
                   enable_asserts=True, num_devices=N_CORES)
    feat = nc.dram_tensor("feat", [3, 2, 128, 18, 90], F32R, kind="ExternalInput").ap()
    w1 = nc.dram_tensor("w1", [2, 2, 128, 9 * 256], F32R, kind="ExternalInput").ap()
    b1 = nc.dram_tensor("b1", [128, 4], F32, kind="ExternalInput").ap()
    w2d = nc.dram_tensor("w2d", [2, 128, DB], F32R, kind="ExternalInput").ap()
    w2c = nc.dram_tensor("w2c", [2, 128, CTX], F32R, kind="ExternalInput").ap()
    b2d = nc.dram_tensor("b2d", [DB, 1], F32, kind="ExternalInput").ap()
    b2c = nc.dram_tensor("b2c", [CTX, 1], F32, kind="ExternalInput").ap()
    dl_out = nc.dram_tensor("dl_out", [3, DB, 16, 88], F32, kind="ExternalOutput").ap()
    ctx_out = nc.dram_tensor("ctx_out", [3, CTX, 16, 88], F32, kind="ExternalOutput").ap()

    with tile.TileContext(nc) as tc:
        with tc.tile_pool(name="wp", bufs=1) as wp, \
             tc.tile_pool(name="fp", bufs=2) as fp, \
             tc.tile_pool(name="hp", bufs=2) as hp, \
             tc.tile_pool(name="op", bufs=2) as op, \
             tc.tile_pool(name="ps", bufs=6, space="PSUM") as ps, \
             tc.tile_pool(name="ps2", bufs=2, space="PSUM") as ps2:

            # feat for half 0 first so the first matmul group starts early
            ft0 = []
            for kt in range(2):
                t_ = fp.tile([128, 18, 90], F32R, tag=f"f{kt}", name=f"f{kt}")
                nc.sync.dma_start(t_[:, 0:8, :], feat[0, kt, :, 0:8, :])
                nc.sync.dma_start(t_[:, 8:18, :], feat[0, kt, :, 8:18, :])
                ft0.append(t_)
            w1ts = {}
            for br in range(2):
                for kt in range(2):
                    t_ = wp.tile([128, 9 * 256], F32R, tag=f"w1{br}{kt}",
                                 name=f"w1{br}{kt}")
                    for c3 in range(3):
                        nc.scalar.dma_start(t_[:, c3 * 768:(c3 + 1) * 768],
                                            w1[br, kt, :, c3 * 768:(c3 + 1) * 768])
                    w1ts[br, kt] = t_
            w2dt = wp.tile([128, 2, DB], F32R, tag="w2d")
            w2ct = wp.tile([128, 2, CTX], F32R, tag="w2c")
            for kt in range(2):
                nc.sync.dma_start(w2dt[:, kt, :], w2d[kt, :, :])
                nc.sync.dma_start(w2ct[:, kt, :], w2c[kt, :, :])
            b1t = wp.tile([128, 4], F32, tag="b1")
            nc.sync.dma_start(b1t[:], b1[:])
            b2dt = wp.tile([DB, 1], F32, tag="b2d")
            nc.sync.dma_start(b2dt[:], b2d[:])
            b2ct = wp.tile([CTX, 1], F32, tag="b2c")
            nc.sync.dma_start(b2ct[:], b2c[:])

            for half in range(3):
                if half == 0:
                    ft = ft0
                else:
                    ft = []
                    for kt in range(2):
                        t_ = fp.tile([128, 18, 90], F32R, tag=f"f{kt}",
                                     name=f"f{kt}")
                        nc.sync.dma_start(t_[:], feat[half, kt, :, :, :])
                        ft.append(t_)
                for br in range(2):
                    ht = [hp.tile([128, 16, 88], F32R, tag=f"h{br}{m}", name=f"h{br}{m}")
                          for m in range(2)]
                    if br == 0:
                        w2t, b2t, no, dst = w2dt, b2dt, DB, dl_out
                    else:
                        w2t, b2t, no, dst = w2ct, b2ct, CTX, ctx_out
                    st = op.tile([no, 16, 88], F32, tag=f"o{br}", name=f"o{br}")
                    for chunk in range(4):
                        for m in range(2):
                            pst = ps.tile([128, 4, 88], F32, tag="acc")
                            n = 0
                            for kt in range(2):
                                for dy in range(3):
                                    for dx in range(3):
                                        nc.tensor.matmul(
                                            pst[:],
                                            w1ts[br, kt][:,
                                                (dy * 3 + dx) * 256 + m * 128:
                                                (dy * 3 + dx) * 256 + m * 128 + 128],
                                            ft[kt][:, chunk * 4 + dy:chunk * 4 + dy + 4,
                                                   dx:dx + 88],
                                            start=(n == 0), stop=(n == 17))
                                        n += 1
                            nc.scalar.activation(
                                ht[m][:, chunk * 4:chunk * 4 + 4, :], pst[:],
                                AF.Relu, bias=b1t[:, br * 2 + m:br * 2 + m + 1])
                        p2 = ps2.tile([128, 4, 88], F32, tag="acc2")
                        for kt in range(2):
                            nc.tensor.matmul(
                                p2[0:no, :, :], w2t[:, kt, :],
                                ht[kt][:, chunk * 4:chunk * 4 + 4, :],
                                start=(kt == 0), stop=(kt == 1))
                        nc.scalar.activation(
                            st[:, chunk * 4:chunk * 4 + 4, :], p2[0:no, :, :],
                            AF.Identity, bias=b2t[:])
                    nc.sync.dma_start(dst[half], st[:])
    nc.compile()
    return nc

# ---------------------------------------------------------------- host prep --
def _fold_bn(w, b, gamma, beta, mean, var, eps=1e-5):
    sc = gamma / np.sqrt(var + eps)
    return (w * sc[:, None, None, None]).astype(np.float32), \
           (b * sc + (beta - mean * sc)).astype(np.float32)

def _phase1_inputs(features, dw1, db1, dgamma, dbeta, dmean, dvar, dw2, db2,
                   cw1, cb1, cgamma, cbeta, cmean, cvar, cw2, cb2):
    dw1f, db1f = _fold_bn(dw1, db1, dgamma, dbeta, dmean, dvar)
    cw1f, cb1f = _fold_bn(cw1, cb1, cgamma, cbeta, cmean, cvar)

    w1 = np.zeros((2, 2, 128, 9 * 256), np.float32)
    for br, wf in ((0, dw1f), (1, cw1f)):
        # lhsT[c_in, o] per tap; layout [kt, c, tap*256 + o]
        t_ = wf.transpose(1, 2, 3, 0).reshape(256, 9, 256)  # [c_in, tap, o]
        for kt in range(2):
            w1[br, kt] = t_[kt * 128:(kt + 1) * 128].reshape(128, 9 * 256)
    b1 = np.zeros((128, 4), np.float32)
    for br, bf in ((0, db1f), (1, cb1f)):
        for m in range(2):
            b1[:, br * 2 + m] = bf[m * 128:(m + 1) * 128]
    w2dh = dw2[:, :, 0, 0].T.reshape(2, 128, DB).astype(np.float32)
    w2ch = cw2[:, :, 0, 0].T.reshape(2, 128, CTX).astype(np.float32)

    feat_cores = []
    for c in range(N_CORES):
        fw = np.zeros((3, 256, 18, 90), np.float32)
        for h in range(3):
            hidx = 3 * c + h
            img, hf = hidx // 2, hidx % 2
            r0 = hf * 16 - 1
            lo, hi = max(r0, 0), min(r0 + 18, 32)
            fw[h, :, lo - r0:hi - r0, 1:89] = features[img, :, lo:hi, :]
        feat_cores.append(fw.reshape(3, 2, 128, 18, 90))

    shared = {"w1": _round_f32r(w1), "b1": b1, "w2d": _round_f32r(w2dh),
              "w2c": _round_f32r(w2ch),
              "b2d": db2.reshape(DB, 1).astype(np.float32),
              "b2c": cb2.reshape(CTX, 1).astype(np.float32)}
    return [{**shared, "feat": _round_f32r(feat_cores[c])}
            for c in range(N_CORES)]

def _phase1_assemble(results):
    dl = np.zeros((B6, DB, H, W), np.float32)
    ctx = np.zeros((B6, CTX, H, W), np.float32)
    for c in range(N_CORES):
        for h in range(3):
            hidx = 3 * c + h
            img, hf = hidx // 2, hidx % 2
            dl[img, :, hf * 16:(hf + 1) * 16] = results[c]["dl_out"][h]
            ctx[img, :, hf * 16:(hf + 1) * 16] = results[c]["ctx_out"][h]
    return dl, ctx

# ------------------------------------------------------------------ phase 2 --
F16 = mybir.dt.float16

def build_phase2():
    nslot_max = max(len(s) for s in B_SLOTS)
    nc = bacc.Bacc("TRN2", target_bir_lowering=False, debug=False,
                   enable_asserts=True, num_devices=N_CORES)
    pw = nc.dram_tensor("pw", [88, NJ, CB], F32R, kind="ExternalInput").ap()
    wwt = nc.dram_tensor("wwt", [88, 402], F32R, kind="ExternalInput").ap()
    wpq = nc.dram_tensor("wpq", [128, 3, 2, 3, 128], F32R, kind="ExternalInput").ap()
    w2b = nc.dram_tensor("w2b", [128, 128], F32R, kind="ExternalInput").ap()
    wb = nc.dram_tensor("wb", [128, TOT_BSLOTS * 128], F32R, kind="ExternalInput").ap()
    tv = nc.dram_tensor("tv", [128, len(INTERIOR)], F32, kind="ExternalInput").ap()
    bb1 = nc.dram_tensor("bb1", [128, 1], F32, kind="ExternalInput").ap()
    bb2 = nc.dram_tensor("bb2", [128, 1], F32, kind="ExternalInput").ap()
    yout = nc.dram_tensor("yout", [O1, 100, BEV], F32, kind="ExternalOutput").ap()

    II = {i: idx for idx, (i, _) in enumerate(INTERIOR)}

    with tile.TileContext(nc) as tc:
        with tc.tile_pool(name="cp", bufs=1) as cp, \
             tc.tile_pool(name="hpool", bufs=1) as hpool, \
             tc.tile_pool(name="sp", bufs=3) as sp, \
             tc.tile_pool(name="wbp", bufs=8) as wbp, \
             tc.tile_pool(name="ps", bufs=5, space="PSUM") as ps, \
             tc.tile_pool(name="ps1", bufs=3, space="PSUM") as ps1:

            # hrow inputs on the ACT HWDGE queue (first), the rest on SP
            wwtt = cp.tile([88, 402], F32R, tag="wwt")
            nc.scalar.dma_start(wwtt[:], wwt[:])
            pwt = cp.tile([88, NJ, CB], F32R, tag="pw")
            for lo, hi in ((0, 2), (2, 7), (7, 12)):
                nc.scalar.dma_start(pwt[:, lo:hi, :], pw[:, lo:hi, :])
            wpqt = cp.tile([128, 3, 2, 3, 128], F32R, tag="wpq")
            nc.sync.dma_start(wpqt[:], wpq[:])
            w2bt = cp.tile([128, 128], F32R, tag="w2b")
            nc.sync.dma_start(w2bt[:], w2b[:])
            tvt = cp.tile([128, len(INTERIOR)], F32, tag="tv")
            nc.sync.dma_start(tvt[:], tv[:])
            bb1t = cp.tile([128, 1], F32, tag="bb1")
            nc.sync.dma_start(bb1t[:], bb1[:])
            bb2t = cp.tile([128, 1], F32, tag="bb2")
            nc.sync.dma_start(bb2t[:], bb2[:])

            hr = [hpool.tile([128, NJ, 402], F32R, tag="hr0", name="hr0"),
                  hpool.tile([64, NJ, 402], F32R, tag="hr1", name="hr1")]
            for jr in range(NJ):
                for kt in range(2):
                    m = 128 if kt == 0 else 64
                    ph = ps.tile([128, 402], F32, tag="acc", name="ph")
                    nc.tensor.matmul(ph[0:m, :],
                                     pwt[:, jr, kt * 128:kt * 128 + m],
                                     wwtt[:], start=True, stop=True)
                    nc.scalar.copy(hr[kt][:, jr, :], ph[0:m, :])
            dh = [hpool.tile([128, NJ - 1, 402], F32R, tag="dh0", name="dh0"),
                  hpool.tile([64, NJ - 1, 402], F32R, tag="dh1", name="dh1")]
            for jr in range(NJ - 1):
                for kt in range(2):
                    nc.vector.tensor_tensor(dh[kt][:, jr, :], hr[kt][:, jr + 1, :],
                                            hr[kt][:, jr, :], ALU.subtract)

            def finish_pair(i, src):
                gt = sp.tile([128, 400], F32R, tag="gt", name="gt")
                nc.scalar.activation(gt[:], src, AF.Gelu, bias=bb1t[:])
                p1_ = ps1.tile([128, 400], F32, tag="acc1", name="p1_")
                nc.tensor.matmul(p1_[:], w2bt[:], gt[:], start=True, stop=True)
                ot = sp.tile([128, 400], F32, tag="ot", name="ot")
                nc.vector.tensor_scalar(ot[:], p1_[:], bb2t[:], None, ALU.add)
                nc.scalar.dma_start(
                    yout[:, 2 * i:2 * i + 2, :].rearrange("o g x -> g o x"), ot[:])

            def do_boundary(bi):
                i, js = BOUNDARY[bi]
                nsl = len(B_SLOTS[bi])
                wbt = wbp.tile([128, nslot_max * 128], F32R, tag="wbt", name="wbt")
                nc.sync.dma_start(wbt[:, 0:nsl * 128],
                                  wb[:, B_OFF[bi] * 128:B_OFF[bi + 1] * 128])
                psB = ps.tile([128, 400], F32, tag="acc", name="psB")
                for si, (kt, t, j) in enumerate(B_SLOTS[bi]):
                    m = 128 if kt == 0 else 64
                    nc.tensor.matmul(psB[:], wbt[0:m, si * 128:si * 128 + 128],
                                     hr[kt][:, j + 1, t:t + 400],
                                     start=(si == 0), stop=(si == nsl - 1))
                finish_pair(i, psB[:])

            bq = list(range(len(BOUNDARY)))

            def do_cell_mms(a):
                ar = a + 1
                psP = ps.tile([128, 400], F32, tag="acc", name="psP")
                n = 0
                for sidx, jr in ((0, ar), (1, ar + 1)):
                    for t in range(3):
                        for kt in range(2):
                            m = 128 if kt == 0 else 64
                            nc.tensor.matmul(psP[:], wpqt[0:m, sidx, kt, t, :],
                                             hr[kt][:, jr, t:t + 400],
                                             start=(n == 0), stop=(n == 11))
                            n += 1
                psQ = ps.tile([128, 400], F32, tag="acc", name="psQ")
                n = 0
                for t in range(3):
                    for kt in range(2):
                        m = 128 if kt == 0 else 64
                        nc.tensor.matmul(psQ[:], wpqt[0:m, 2, kt, t, :],
                                         dh[kt][:, ar, t:t + 400],
                                         start=(n == 0), stop=(n == 5))
                        n += 1
                Pt = sp.tile([128, 400], F32R, tag="Pt", name="Pt")
                nc.scalar.copy(Pt[:], psP[:])
                Qt = sp.tile([128, 400], F32R, tag="Qt", name="Qt")
                nc.vector.tensor_copy(Qt[:], psQ[:])
                return Pt, Qt

            def do_cell_pairs(a, Pt, Qt):
                for i in PAIRS_OF_CELL[a]:
                    idx = II[i]
                    yt = sp.tile([128, 400], F32R, tag="yt", name="yt")
                    nc.vector.tensor_scalar(yt[:], Qt[:], tvt[:, idx:idx + 1],
                                            None, ALU.mult)
                    nc.vector.tensor_tensor(yt[:], yt[:], Pt[:], ALU.add)
                    finish_pair(i, yt[:])

            # one-stage software pipeline: next cell's MMs before this cell's
            # row-finishes, boundary groups as PE filler in between
            prev = None
            for ci, a in enumerate(CELLS):
                pq = do_cell_mms(a)
                if prev is not None:
                    if ci >= 1 and bq:
                        do_boundary(bq.pop(0))
                    if ci >= 3 and bq:
                        do_boundary(bq.pop(0))
                    do_cell_pairs(*prev)
                prev = (a, *pq)
            if prev is not None:
                do_cell_pairs(prev[0], prev[1], prev[2])
            while bq:
                do_boundary(bq.pop(0))
    nc.compile()
    return nc

def _coef(rr_abs, j_abs):
    if rr_abs < 0 or rr_abs >= BEV:
        return 0.0
    s = (rr_abs + 0.5) * 0.08 - 0.5
    a = int(np.floor(s)); t = s - a
    return (1 - t) if j_abs == a else (t if j_abs == a + 1 else 0.0)

def _phase2_inputs(pooled, bw1, bb1, bw2, bb2):
    Ww = _interp_matrix(W, BEV)
    wwt = np.zeros((88, 402), np.float32)
    wwt[:, 1:401] = Ww.T

    Wsum = bw1.sum(axis=2)                    # (O1, CB, 3)
    Wdiff = bw1[:, :, 2] - bw1[:, :, 0]
    Wsets = [Wsum - 0.08 * Wdiff, 0.08 * Wdiff, Wsum]
    wpq = np.zeros((3, 2, 3, 128, 128), np.float32)
    for s_ in range(3):
        for kt in range(2):
            m = 128 if kt == 0 else 64
            for t in range(3):
                blk = Wsets[s_][:, kt * 128:kt * 128 + m, t]   # (O1, m)
                for g in range(2):
                    wpq[s_, kt, t, 0:m, g * 64:g * 64 + 64] = blk.T
    w2b = np.zeros((128, 128), np.float32)
    for g in range(2):
        w2b[g * 64:g * 64 + 64, g * 64:g * 64 + 64] = bw2[:, :, 0, 0].T
    tv = np.zeros((128, len(INTERIOR)), np.float32)
    for idx, (i, a) in enumerate(INTERIOR):
        for g in range(2):
            tv[g * 64:(g + 1) * 64, idx] = (2 * i + g + 0.5) * 0.08 - 0.5 - a
    bb1d = np.tile(bb1, 2).reshape(128, 1).astype(np.float32)
    bb2d = np.tile(bb2, 2).reshape(128, 1).astype(np.float32)

    shared = {"wwt": _round_f32r(wwt),
              "wpq": _round_f32r(wpq.transpose(3, 0, 1, 2, 4).copy()),
              "w2b": _round_f32r(w2b), "tv": tv, "bb1": bb1d, "bb2": bb2d}

    in_maps = []
    for c in range(N_CORES):
        bat, blk = c // 4, c % 4
        pwin = np.zeros((88, NJ, CB), np.float32)
        for jr in range(NJ):
            j = int(np.clip(8 * blk - 1 + jr, 0, H - 1))
            pwin[:, jr, :] = pooled[bat, :, j, :].T
        wbv = np.zeros((128, TOT_BSLOTS * 128), np.float32)
        for bi, (i, js) in enumerate(BOUNDARY):
            for si, (kt, t, j) in enumerate(B_SLOTS[bi]):
                gs = B_OFF[bi] + si
                m = 128 if kt == 0 else 64
                for g in range(2):
                    W_ = np.zeros((O1, m), np.float32)
                    for d in range(3):
                        cf = _coef(100 * blk + 2 * i + g + d - 1, 8 * blk + j)
                        if cf != 0.0:
                            W_ += cf * bw1[:, kt * 128:kt * 128 + m, d, t]
                    wbv[0:m, gs * 128 + g * 64:gs * 128 + g * 64 + 64] = W_.T
        in_maps.append({**shared, "pw": _round_f32r(pwin),
                        "wb": _round_f32r(wbv)})
    return in_maps

def _phase2_assemble(results):
    out = np.zeros((2, O1, BEV, BEV), np.float32)
    for c in range(N_CORES):
        bat, blk = c // 4, c % 4
        out[bat, :, 100 * blk:100 * blk + 100, :] = results[c]["yout"]
    return out

# ------------------------------------------------------------------- cache --
_NC_CACHE = {}

def _get_nc(name, builder):
    if name not in _NC_CACHE:
        _NC_CACHE[name] = builder()
    return _NC_CACHE[name]

def kernel(**inputs):
    inputs = {k: np.asarray(v) for k, v in inputs.items()}
    p1_keys = ['features', 'dw1', 'db1', 'dgamma', 'dbeta', 'dmean', 'dvar',
               'dw2', 'db2', 'cw1', 'cb1', 'cgamma', 'cbeta', 'cmean', 'cvar',
               'cw2', 'cb2']
    nc1 = _get_nc("p1", build_phase1)
    in_maps = _phase1_inputs(*[inputs[k] for k in p1_keys])
    res1 = bass_utils.run_bass_kernel_spmd(nc1, in_maps,
                                           core_ids=list(range(N_CORES)))
    dl, ctx = _phase1_assemble(res1.results)
    pooled = ctx.reshape(2, 6 * CTX, H, W)

    nc2 = _get_nc("p2", build_phase2)
    in_maps2 = _phase2_inputs(pooled, inputs['bw1'], inputs['bb1'],
                              inputs['bw2'], inputs['bb2'])
    res2 = bass_utils.run_bass_kernel_spmd(nc2, in_maps2,
                                           core_ids=list(range(N_CORES)))
    out = _phase2_assemble(res2.results)
    return out, dl
